# revision 1
# baseline (speedup 1.0000x reference)
"""Trainium2 Bass kernel for nn_DeformKernelConv2d.

Math (per batch image; shapes below are per core after sharding):
  offsets:  off = conv3x3(x, offset_w) + offset_b          -> dy,dx per (k, pixel)
  coords:   yc_k = dy_k + by_k ; xc_k = dx_k + bx_k        (scope-kernel space)
  phi:      phi_y[k,i] = relu(1-|yc_k - i|), i=0..3        (likewise phi_x)
  Phi:      Phi_k[4*yi+xi] = phi_y[k,yi] * phi_x[k,xi]     (bilinear weights, 16 per k)
  samp:     samp_k[c] = sum_s wflat[c,s] * Phi_k[s]        (matmul over s=16)
  out:      out[c] = sum_k samp_k[c] * x_k[c]              (x_k = 3x3-shifted x)

Device mapping:
  - 8 cores: (batch b, H-half); each core does 28 rows with a 1-row halo.
  - The offset conv is fused with the affine (coord - grid line i) expansion
    into one 9-tap accumulated matmul producing T[72, pix] (rows (k, axis, i)),
    with per-row bias = offset_b + base - i folded into the Abs activation.
  - phi on ScalarE (Abs then Relu), phi->Y/X row replication by DMA,
    Phi products + final MAC on VectorE, samp matmuls row-packed 4x on PE.
"""

import numpy as np
import ml_dtypes

B, C, H, W = 4, 128, 56, 56
HC = H // 2            # 28 rows per core
NPIX = HC * W          # 1568
CH = 7                 # chunk height (rows)
NCH = HC // CH         # 4 chunks
CHN = CH * W           # 392 columns per chunk
RA, RB = 58, 60        # padded row lengths: xbfA data at col 1, xbfB at col 2

_BF16 = ml_dtypes.bfloat16
_cache = {}


def _build_program(repeat=1):
    import concourse.tile as tile
    import concourse.mybir as mybir
    from concourse import bacc

    fp32 = mybir.dt.float32
    bf16 = mybir.dt.bfloat16
    AF = mybir.ActivationFunctionType

    nc = bacc.Bacc("TRN2", target_bir_lowering=False, debug=False, num_devices=8)
    xs_d = nc.dram_tensor("xs", [C, HC + 2, W], fp32, kind="ExternalInput")
    lhsT_d = nc.dram_tensor("lhsT", [C, 9 * 72], bf16, kind="ExternalInput")
    w4T_d = nc.dram_tensor("w4T", [C, C], bf16, kind="ExternalInput")
    bias_d = nc.dram_tensor("bias72", [72, 1], fp32, kind="ExternalInput")
    out_d = nc.dram_tensor("out", [C, HC, W], fp32, kind="ExternalOutput")

    with tile.TileContext(nc) as tc:
        with (
            tc.tile_pool(name="const", bufs=1) as cp,
            tc.tile_pool(name="work", bufs=1) as wp,
            tc.tile_pool(name="tmp", bufs=3) as tp,
            tc.tile_pool(name="psT", bufs=2, space="PSUM") as ppT,
            tc.tile_pool(name="psS", bufs=5, space="PSUM") as ppS,
        ):
            lhsT = cp.tile([C, 9 * 72], bf16)
            nc.sync.dma_start(lhsT[:], lhsT_d[:])
            w4T = cp.tile([C, C], bf16)
            nc.sync.dma_start(w4T[:], w4T_d[:])
            bias = cp.tile([72, 1], fp32)
            nc.sync.dma_start(bias[:], bias_d[:])

            xbfA = cp.tile([C, HC + 2, RA], bf16)
            xbfB = cp.tile([C, HC + 2, RB], bf16)
            nc.vector.memset(xbfA[:, :, 0:1], 0)
            nc.vector.memset(xbfA[:, :, 57:58], 0)
            nc.vector.memset(xbfB[:, :, 0:2], 0)
            nc.vector.memset(xbfB[:, :, 58:60], 0)

            phi = wp.tile([72, NCH, CHN], bf16)
            Ya = wp.tile([C, NCH, CHN], bf16)
            Xa = wp.tile([C, NCH, CHN], bf16)
            Yb = wp.tile([C, NCH, CHN], bf16)
            Xb = wp.tile([C, NCH, CHN], bf16)
            Yc = wp.tile([16, NCH, CHN], bf16)
            Xc = wp.tile([16, NCH, CHN], bf16)
            PhA = wp.tile([C, NCH, CHN], bf16)
            PhB = wp.tile([C, NCH, CHN], bf16)
            PhC = wp.tile([16, NCH, CHN], bf16)
            samp = wp.tile([C, 9, NPIX], bf16)
            prod = wp.tile([C, 9, NPIX], bf16)
            t1 = wp.tile([C, 4, NPIX], bf16)
            t2 = wp.tile([C, 2, NPIX], bf16)
            t3 = wp.tile([C, NPIX], bf16)
            res = wp.tile([C, NPIX], fp32)

            # replication views (single strided partition dim per DMA side —
            # multi-dim partition APs mislower in the DMA path)
            phiV = phi[:].rearrange("(k a i) c n -> k a i c n", k=9, a=2, i=4)

            def rep_view(t):
                return t[:].rearrange("(g h y x) c n -> g h y x c n", h=2, y=4, x=4)

            YaV, XaV, YbV, XbV = rep_view(Ya), rep_view(Xa), rep_view(Yb), rep_view(Xb)
            YcV = Yc[:].rearrange("(y x) c n -> y x c n", x=4)
            XcV = Xc[:].rearrange("(y x) c n -> y x c n", x=4)

            for _rep in range(repeat):
                nc.gpsimd.dma_start(xbfA[:, :, 1:57], xs_d[:])  # casts fp32->bf16
                nc.gpsimd.dma_start(xbfB[:, :, 2:58], xs_d[:])

                # ---- T matmul (offset conv + affine expansion) + phi ----
                for ch in range(NCH):
                    psT = ppT.tile([72, CHN], fp32, tag="psT")
                    for tap in range(9):
                        di, dj = tap // 3, tap % 3
                        rhs = xbfA[:, ch * CH + di : ch * CH + di + CH, dj : dj + W]
                        nc.tensor.matmul(
                            psT[:],
                            lhsT[:, tap * 72 : (tap + 1) * 72],
                            rhs,
                            start=(tap == 0),
                            stop=(tap == 8),
                        )
                    u = tp.tile([72, CHN], fp32, tag="u")
                    nc.scalar.activation(u[:], psT[:], AF.Abs, bias=bias[:], scale=1.0)
                    nc.scalar.activation(
                        phi[:, ch, :], u[:], AF.Relu, bias=1.0, scale=-1.0
                    )

                # ---- replicate phi rows into Y/X s-patterns (72 DMAs) ----
                for g in range(4):
                    for xi in range(4):
                        nc.sync.dma_start(YaV[g, 0, :, xi], phiV[g, 0, :])
                        nc.sync.dma_start(YbV[g, 0, :, xi], phiV[4 + g, 0, :])
                    for yi in range(4):
                        nc.sync.dma_start(XaV[g, 0, yi, :], phiV[g, 1, :])
                        nc.sync.dma_start(XbV[g, 0, yi, :], phiV[4 + g, 1, :])
                for xi in range(4):
                    nc.sync.dma_start(YcV[:, xi], phiV[8, 0, :])
                for yi in range(4):
                    nc.sync.dma_start(XcV[yi, :], phiV[8, 1, :])

                # ---- Phi products, samp matmuls (row-packed), PSUM drain ----
                for ch in range(NCH):
                    nc.vector.tensor_mul(PhA[:, ch, :], Ya[:, ch, :], Xa[:, ch, :])
                    nc.vector.tensor_mul(PhB[:, ch, :], Yb[:, ch, :], Xb[:, ch, :])
                    nc.vector.tensor_mul(PhC[:, ch, :], Yc[:, ch, :], Xc[:, ch, :])
                    for k in range(9):
                        g = k % 4
                        src = (PhA, PhB, PhC)[k // 4]
                        base = 32 * g if k < 8 else 0
                        psS = ppS.tile([C, CHN], fp32, tag="psS")
                        nc.tensor.matmul(
                            psS[:],
                            w4T[base : base + 16, :],
                            src[base : base + 16, ch, :],
                            start=True,
                            stop=True,
                            tile_position=(base, 0),
                        )
                        dst = samp[:, k, ch * CHN : (ch + 1) * CHN]
                        if k < 6:
                            nc.scalar.copy(dst, psS[:])
                        else:
                            nc.vector.tensor_copy(out=dst, in_=psS[:])

                # ---- products with shifted x, tree sum, store ----
                for k in range(9):
                    di, dj = k // 3, k % 3
                    if dj == 1:
                        xsrc, coff = xbfB, dj + 1  # col offset 2: 4B aligned
                    else:
                        xsrc, coff = xbfA, dj  # col offsets 0, 2
                    xv = xsrc[:, di : di + HC, coff : coff + W]
                    nc.vector.tensor_mul(
                        prod[:, k, :].rearrange("p (h w) -> p h w", h=HC),
                        samp[:, k, :].rearrange("p (h w) -> p h w", h=HC),
                        xv,
                    )
                nc.vector.tensor_add(t1[:], prod[:, 0:4, :], prod[:, 4:8, :])
                nc.vector.tensor_add(t2[:], t1[:, 0:2, :], t1[:, 2:4, :])
                nc.vector.tensor_add(t3[:], t2[:, 0, :], t2[:, 1, :])
                nc.vector.tensor_add(res[:], t3[:], prod[:, 8, :])
                nc.sync.dma_start(
                    out_d[:], res[:].rearrange("p (h w) -> p h w", h=HC)
                )

    nc.finalize()
    return nc


def _prep_inputs(x, offset_w, offset_b, weight):
    """Host-side sharding + weight reshaping. Returns per-core input maps."""
    x = np.asarray(x, dtype=np.float32)
    offset_w = np.asarray(offset_w, dtype=np.float32)
    offset_b = np.asarray(offset_b, dtype=np.float32)
    weight = np.asarray(weight, dtype=np.float32)

    # lhsT[c, tap*72 + k*8 + axis*4 + i] = offset_w[2k+axis, c, tap//3, tap%3]
    ow = offset_w.reshape(9, 2, C, 3, 3)  # [k, axis, c, di, dj]
    lhsT = np.transpose(ow, (2, 3, 4, 0, 1))  # [c, di, dj, k, axis]
    lhsT = np.repeat(lhsT[..., None], 4, axis=-1)  # [c, di, dj, k, axis, i]
    lhsT = np.ascontiguousarray(lhsT.reshape(C, 648)).astype(_BF16)

    # w4T rows 32g+s = weight[:, s//4, s%4]
    w4T = np.zeros((C, C), dtype=_BF16)
    wT = weight.reshape(C, 16).T.astype(_BF16)  # [16, C]
    for g in range(4):
        w4T[32 * g : 32 * g + 16, :] = wT

    # bias72[k*8+axis*4+i] = offset_b[2k+axis] + base - i
    base = np.arange(3, dtype=np.float32) + 0.5
    bias = np.zeros((9, 2, 4), dtype=np.float32)
    for k in range(9):
        for axis in range(2):
            bv = base[k // 3] if axis == 0 else base[k % 3]
            bias[k, axis, :] = offset_b[2 * k + axis] + bv - np.arange(4)
    bias72 = bias.reshape(72, 1)

    in_maps = []
    for core in range(8):
        b, half = core // 2, core % 2
        h0 = half * HC
        xs = np.zeros((C, HC + 2, W), dtype=np.float32)
        lo, hi = h0 - 1, h0 + HC + 1
        slo, shi = max(lo, 0), min(hi, H)
        xs[:, slo - lo : slo - lo + (shi - slo), :] = x[b, :, slo:shi, :]
        in_maps.append({"xs": xs, "lhsT": lhsT, "w4T": w4T, "bias72": bias72})
    return in_maps


def kernel(x, offset_w, offset_b, weight):
    from concourse.bass_utils import run_bass_kernel_spmd

    if "nc" not in _cache:
        _cache["nc"] = _build_program()
    nc = _cache["nc"]

    in_maps = _prep_inputs(x, offset_w, offset_b, weight)
    res = run_bass_kernel_spmd(nc, in_maps, core_ids=list(range(8)))

    out = np.zeros((B, C, H, W), dtype=np.float32)
    for core in range(8):
        b, half = core // 2, core % 2
        out[b, :, half * HC : (half + 1) * HC, :] = res.results[core]["out"].reshape(
            C, HC, W
        )
    return out



# revision 2
# speedup vs baseline: 1.6032x; 1.6032x over previous
"""Trainium2 Bass kernel for nn_DeformKernelConv2d.

Math (per batch image; shapes below are per core after sharding):
  offsets:  off = conv3x3(x, offset_w) + offset_b          -> dy,dx per (k, pixel)
  coords:   yc_k = dy_k + by_k ; xc_k = dx_k + bx_k        (scope-kernel space)
  phi:      phi_y[k,i] = relu(1-|yc_k - i|), i=0..3        (likewise phi_x)
  Phi:      Phi_k[4*yi+xi] = phi_y[k,yi] * phi_x[k,xi]     (bilinear weights, 16 per k)
  samp:     samp_k[c] = sum_s wflat[c,s] * Phi_k[s]        (matmul over s=16)
  out:      out[c] = sum_k samp_k[c] * x_k[c]              (x_k = 3x3-shifted x)

Device mapping (v2):
  - 8 cores: (batch b, H-half); each core does 28 rows with a 1-row halo.
  - offset conv fused with the affine (coord - grid line i) expansion into a
    9-tap accumulated matmul -> T[72, pix], rows (k, axis, i).
  - u = |T + bias| via one ACT op (bias folds offset_b + grid).
  - Row replication of u into the 128-row (k, s) patterns is done with PE
    matmuls against constant 0/1 selection matrices (S_Ay/S_Ax/S_By/S_Bx),
    not DMAs.  k=8's patterns ride in rows 16..31 of the A tile (the spare
    half-strips); its samp matmul contracts rows 0..31 against a [32,128]
    weight whose top half is zero.
  - phi = relu(1 - u) fuses into the PSUM->SBUF drain of the replication
    matmuls (ACT, bias=1, scale=-1).
  - Phi = phi_y * phi_x on DVE (bf16 SBUF, 2x mode).
  - samp matmuls: contract 16 (32 for k=8) with tile_position quadrants.
  - tail (prod with shifted x + 9-way k-sum) split into two pixel halves,
    work spread across DVE and GpSimd.
  - x is pre-cast to bf16 and pre-padded on the host (two layouts A/B), so
    there are no on-device casts or memsets.
"""

import numpy as np
import ml_dtypes

B, C, H, W = 4, 128, 56, 56
HC = H // 2            # 28 rows per core
NPIX = HC * W          # 1568
CH = 7                 # chunk height (rows)
NCH = HC // CH         # 4 chunks
CHN = CH * W           # 392 columns per chunk
RA, RB = 58, 60        # padded row lengths: xbfA data at col 1, xbfB at col 2
NHALF = NPIX // 2      # 784 (14 rows) per tail half

_BF16 = ml_dtypes.bfloat16
_cache = {}

# number of samp drains routed to the scalar engine per chunk (rest on DVE)
ACT_DRAINS = 7


def _build_program():
    import concourse.tile as tile
    import concourse.mybir as mybir
    from concourse import bacc

    fp32 = mybir.dt.float32
    bf16 = mybir.dt.bfloat16
    AF = mybir.ActivationFunctionType

    nc = bacc.Bacc("TRN2", target_bir_lowering=False, debug=False, num_devices=8)
    # consts cols: [0:648] lhsT | [648:776] w4T | [776:904] w8T(rows<32)
    #             | [904:1160] SA (Ay|Ax) | [1160:1416] SB (By|Bx)
    consts_d = nc.dram_tensor("consts", [C, 1416], bf16, kind="ExternalInput")
    bias_d = nc.dram_tensor("bias72", [72, 1], fp32, kind="ExternalInput")
    xsA_d = nc.dram_tensor("xsA", [C, HC + 2, RA], bf16, kind="ExternalInput")
    xsB_d = nc.dram_tensor("xsB", [C, HC + 2, RB], bf16, kind="ExternalInput")
    out_d = nc.dram_tensor("out", [C, HC, W], fp32, kind="ExternalOutput")

    with tile.TileContext(nc) as tc:
        with (
            tc.tile_pool(name="const", bufs=1) as cp,
            tc.tile_pool(name="work", bufs=1) as wp,
            tc.tile_pool(name="tmp", bufs=3) as tp,
            tc.tile_pool(name="psT", bufs=1, space="PSUM") as ppT,
            tc.tile_pool(name="rep", bufs=2, space="PSUM") as ppR,
            tc.tile_pool(name="psS", bufs=3, space="PSUM") as ppS,
        ):
            consts = cp.tile([C, 1416], bf16)
            bias = cp.tile([72, 1], fp32)
            xbfA = cp.tile([C, HC + 2, RA], bf16)
            xbfB = cp.tile([C, HC + 2, RB], bf16)

            lhsT = consts[:, 0:648]
            w4T = consts[:, 648:776]
            w8T = consts[0:32, 776:904]
            SA = consts[0:72, 904:1160]
            SB = consts[0:72, 1160:1416]

            # loads: consts first (gates taps), then x rows for chunk 0,
            # then the rest.
            nc.sync.dma_start(consts[:], consts_d[:])
            nc.sync.dma_start(xbfA[:, 0:9, :], xsA_d[:, 0:9, :])
            nc.sync.dma_start(xbfA[:, 9:30, :], xsA_d[:, 9:30, :])
            nc.sync.dma_start(bias[:], bias_d[:])
            nc.sync.dma_start(xbfB[:], xsB_d[:])

            PhA = wp.tile([C, NCH, CHN], bf16)
            PhB = wp.tile([C, NCH, CHN], bf16)
            samp = wp.tile([C, 9, NPIX], bf16)
            prod = wp.tile([C, 9, NPIX], bf16)
            q1 = wp.tile([C, 2, 2, NHALF], bf16)   # [half][2]
            q2 = wp.tile([C, 2, NHALF], bf16)
            q3 = wp.tile([C, 2, NHALF], bf16)
            g1 = wp.tile([C, 2, 2, NHALF], bf16)   # gpsimd partials
            g2 = wp.tile([C, 2, NHALF], bf16)
            res = wp.tile([C, 2, NHALF], fp32)

            for ch in range(NCH):
                # ---- T matmul (offset conv + affine expansion) ----
                psT = ppT.tile([72, CHN], fp32, tag="psT")
                for tap in range(9):
                    di, dj = tap // 3, tap % 3
                    rhs = xbfA[:, ch * CH + di : ch * CH + di + CH, dj : dj + W]
                    nc.tensor.matmul(
                        psT[:],
                        lhsT[:, tap * 72 : (tap + 1) * 72],
                        rhs,
                        start=(tap == 0),
                        stop=(tap == 8),
                    )
                # u = |T + bias|  (PSUM -> SBUF bf16)
                u = tp.tile([72, CHN], bf16, tag="u")
                nc.scalar.activation(u[:], psT[:], AF.Abs, bias=bias[:], scale=1.0)

                # ---- replication matmuls: u rows -> (k,s) patterns ----
                UA = ppR.tile([C, 2, 512], fp32, tag="rep")
                UB = ppR.tile([C, 2, 512], fp32, tag="rep")
                nc.tensor.matmul(UA[:, 0, 0:CHN], SA[:, 0:128], u[:], start=True, stop=True)
                nc.tensor.matmul(UA[:, 1, 0:CHN], SA[:, 128:256], u[:], start=True, stop=True)
                nc.tensor.matmul(UB[:, 0, 0:CHN], SB[:, 0:128], u[:], start=True, stop=True)
                nc.tensor.matmul(UB[:, 1, 0:CHN], SB[:, 128:256], u[:], start=True, stop=True)

                # phi = relu(1 - u), fused into the drains
                YXA = tp.tile([C, 2, CHN], bf16, tag="yxa")
                YXB = tp.tile([C, 2, CHN], bf16, tag="yxb")
                nc.scalar.activation(YXA[:], UA[:, :, 0:CHN], AF.Relu, bias=1.0, scale=-1.0)
                nc.scalar.activation(YXB[:], UB[:, :, 0:CHN], AF.Relu, bias=1.0, scale=-1.0)

                # ---- Phi products (bf16 SBUF, 2x) ----
                nc.vector.tensor_mul(PhA[:, ch, :], YXA[:, 0, :], YXA[:, 1, :])
                nc.vector.tensor_mul(PhB[:, ch, :], YXB[:, 0, :], YXB[:, 1, :])

                # ---- samp matmuls + drains ----
                for k in range(9):
                    psS = ppS.tile([C, CHN], fp32, tag="psS")
                    if k == 8:
                        nc.tensor.matmul(
                            psS[:], w8T, PhA[0:32, ch, :],
                            start=True, stop=True, tile_position=(0, 0),
                        )
                    else:
                        g = k % 4
                        src = PhA if k < 4 else PhB
                        base = 32 * g
                        nc.tensor.matmul(
                            psS[:],
                            w4T[base : base + 16, :],
                            src[base : base + 16, ch, :],
                            start=True, stop=True, tile_position=(base, 0),
                        )
                    dst = samp[:, k, ch * CHN : (ch + 1) * CHN]
                    if k < ACT_DRAINS:
                        nc.scalar.copy(dst, psS[:])
                    else:
                        nc.vector.tensor_copy(out=dst, in_=psS[:])

                # ---- tail per half (after chunks 1 and 3) ----
                if ch % 2 == 1:
                    h = ch // 2
                    c0, c1 = h * NHALF, (h + 1) * NHALF
                    HH = NHALF // W  # 14 rows
                    r0 = h * HH
                    for k in range(9):
                        di, dj = k // 3, k % 3
                        if dj == 1:
                            xsrc, coff = xbfB, 2
                        else:
                            xsrc, coff = xbfA, dj
                        xv = xsrc[:, r0 + di : r0 + di + HH, coff : coff + W]
                        eng = nc.vector if k < 5 else nc.gpsimd
                        eng.tensor_mul(
                            prod[:, k, c0:c1].rearrange("p (h w) -> p h w", h=HH),
                            samp[:, k, c0:c1].rearrange("p (h w) -> p h w", h=HH),
                            xv,
                        )
                    pv = prod[:, :, c0:c1]
                    # DVE: p0..p4 -> q3[h]
                    nc.vector.tensor_add(q1[:, h], pv[:, 0:2, :], pv[:, 2:4, :])
                    nc.vector.tensor_add(q2[:, h], q1[:, h, 0, :], q1[:, h, 1, :])
                    nc.vector.tensor_add(q3[:, h], q2[:, h], pv[:, 4, :])
                    # GpSimd: p5..p8 -> g2[h]
                    nc.gpsimd.tensor_add(g1[:, h], pv[:, 5:7, :], pv[:, 7:9, :])
                    nc.gpsimd.tensor_add(g2[:, h], g1[:, h, 0, :], g1[:, h, 1, :])
                    # merge + store
                    nc.vector.tensor_add(res[:, h], q3[:, h], g2[:, h])
                    nc.sync.dma_start(
                        out_d[:, r0 : r0 + HH, :],
                        res[:, h].rearrange("p (h w) -> p h w", h=HH),
                    )

    nc.finalize()
    return nc


def _prep_inputs(x, offset_w, offset_b, weight):
    """Host-side sharding + weight reshaping. Returns per-core input maps."""
    x = np.asarray(x, dtype=np.float32)
    offset_w = np.asarray(offset_w, dtype=np.float32)
    offset_b = np.asarray(offset_b, dtype=np.float32)
    weight = np.asarray(weight, dtype=np.float32)

    # lhsT[c, tap*72 + k*8 + axis*4 + i] = offset_w[2k+axis, c, tap//3, tap%3]
    ow = offset_w.reshape(9, 2, C, 3, 3)  # [k, axis, c, di, dj]
    lhsT = np.transpose(ow, (2, 3, 4, 0, 1))  # [c, di, dj, k, axis]
    lhsT = np.repeat(lhsT[..., None], 4, axis=-1)  # [c, di, dj, k, axis, i]
    lhsT = lhsT.reshape(C, 648)

    # w4T rows 32g+s = weight[:, s//4, s%4]; w8T rows 16..31 = same
    wT = weight.reshape(C, 16).T  # [16, C]
    w4T = np.zeros((C, 128), dtype=np.float32)
    for g in range(4):
        w4T[32 * g : 32 * g + 16, :] = wT
    w8T = np.zeros((C, 128), dtype=np.float32)
    w8T[16:32, :] = wT

    # selection matrices [72, 128]: row r = k*8 + axis*4 + i
    SAy = np.zeros((C, 128), dtype=np.float32)
    SAx = np.zeros((C, 128), dtype=np.float32)
    SBy = np.zeros((C, 128), dtype=np.float32)
    SBx = np.zeros((C, 128), dtype=np.float32)
    for k in range(4):
        for s in range(16):
            yi, xi = s // 4, s % 4
            SAy[k * 8 + yi, 32 * k + s] = 1.0
            SAx[k * 8 + 4 + xi, 32 * k + s] = 1.0
            SBy[(k + 4) * 8 + yi, 32 * k + s] = 1.0
            SBx[(k + 4) * 8 + 4 + xi, 32 * k + s] = 1.0
    # k=8 patterns ride in A rows 16..31
    for s in range(16):
        yi, xi = s // 4, s % 4
        SAy[64 + yi, 16 + s] = 1.0
        SAx[64 + 4 + xi, 16 + s] = 1.0

    consts = np.concatenate(
        [lhsT, w4T, w8T, np.concatenate([SAy, SAx], axis=1),
         np.concatenate([SBy, SBx], axis=1)], axis=1
    ).astype(_BF16)

    # bias72[k*8+axis*4+i] = offset_b[2k+axis] + base - i
    base = np.arange(3, dtype=np.float32) + 0.5
    bias = np.zeros((9, 2, 4), dtype=np.float32)
    for k in range(9):
        for axis in range(2):
            bv = base[k // 3] if axis == 0 else base[k % 3]
            bias[k, axis, :] = offset_b[2 * k + axis] + bv - np.arange(4)
    bias72 = bias.reshape(72, 1)

    xb = x.astype(_BF16)
    in_maps = []
    for core in range(8):
        b, half = core // 2, core % 2
        h0 = half * HC
        xsA = np.zeros((C, HC + 2, RA), dtype=_BF16)
        xsB = np.zeros((C, HC + 2, RB), dtype=_BF16)
        lo, hi = h0 - 1, h0 + HC + 1
        slo, shi = max(lo, 0), min(hi, H)
        xsA[:, slo - lo : slo - lo + (shi - slo), 1:57] = xb[b, :, slo:shi, :]
        xsB[:, slo - lo : slo - lo + (shi - slo), 2:58] = xb[b, :, slo:shi, :]
        in_maps.append(
            {"consts": consts, "bias72": bias72, "xsA": xsA, "xsB": xsB}
        )
    return in_maps


def kernel(x, offset_w, offset_b, weight):
    from concourse.bass_utils import run_bass_kernel_spmd

    if "nc" not in _cache:
        _cache["nc"] = _build_program()
    nc = _cache["nc"]

    in_maps = _prep_inputs(x, offset_w, offset_b, weight)
    res = run_bass_kernel_spmd(nc, in_maps, core_ids=list(range(8)))

    out = np.zeros((B, C, H, W), dtype=np.float32)
    for core in range(8):
        b, half = core // 2, core % 2
        out[b, :, half * HC : (half + 1) * HC, :] = res.results[core]["out"].reshape(
            C, HC, W
        )
    return out


# revision 3
# speedup vs baseline: 1.7112x; 1.0674x over previous
"""Trainium2 Bass kernel for nn_DeformKernelConv2d.

Math (per batch image; shapes below are per core after sharding):
  offsets:  off = conv3x3(x, offset_w) + offset_b          -> dy,dx per (k, pixel)
  coords:   yc_k = dy_k + by_k ; xc_k = dx_k + bx_k        (scope-kernel space)
  phi:      phi_y[k,i] = relu(1-|yc_k - i|), i=0..3        (likewise phi_x)
  Phi:      Phi_k[4*yi+xi] = phi_y[k,yi] * phi_x[k,xi]     (bilinear weights, 16 per k)
  samp:     samp_k[c] = sum_s wflat[c,s] * Phi_k[s]        (matmul over s=16)
  out:      out[c] = sum_k samp_k[c] * x_k[c]              (x_k = 3x3-shifted x)

Device mapping (v3):
  - 8 cores: (batch b, H-half); each core does 28 rows with a 1-row halo.
  - offset conv fused with the affine (coord - grid line i) expansion into a
    9-tap accumulated matmul -> T[72, pix], rows (k, axis, i).
  - u = |T + bias| via one ACT op (bias folds offset_b + grid).
  - Row replication of u into the 128-row (k, s) patterns via PE matmuls
    against constant 0/1 selection matrices.  k=8's patterns ride in rows
    16..31 of the A tile; its samp matmul contracts rows 0..31 against a
    [32,128] weight whose top half is zero.
  - phi = relu(1 - u) fuses into the PSUM->SBUF drains (ACT).
  - samp matmuls write pair-packed 2-bank PSUM tiles; drains move 2 k-planes
    per instruction, split between ACT and DVE.
  - tail: per pixel-half, 9 prod muls on DVE; k-sum tree levels 1-2 run as
    SBUF->SBUF accumulate-DMAs on the idle DMA engines, final adds on DVE.
  - x is pre-cast to bf16 and pre-padded on the host (two layouts A/B).
"""

import numpy as np
import ml_dtypes

B, C, H, W = 4, 128, 56, 56
HC = H // 2            # 28 rows per core
NPIX = HC * W          # 1568
CH = 7                 # chunk height (rows)
NCH = HC // CH         # 4 chunks
CHN = CH * W           # 392 columns per chunk
RA, RB = 58, 60        # padded row lengths: xbfA data at col 1, xbfB at col 2
NHALF = NPIX // 2      # 784 (14 rows) per tail half

_BF16 = ml_dtypes.bfloat16
_cache = {}


def _build_program():
    import concourse.tile as tile
    import concourse.mybir as mybir
    from concourse import bacc

    fp32 = mybir.dt.float32
    bf16 = mybir.dt.bfloat16
    AF = mybir.ActivationFunctionType
    ADD = mybir.AluOpType.add

    nc = bacc.Bacc("TRN2", target_bir_lowering=False, debug=False, num_devices=8)
    # consts cols: [0:648] lhsT | [648:776] w4T | [776:904] w8T(rows<32)
    #             | [904:1160] SA (Ay|Ax) | [1160:1416] SB (By|Bx)
    consts_d = nc.dram_tensor("consts", [C, 1416], bf16, kind="ExternalInput")
    bias_d = nc.dram_tensor("bias72", [72, 1], fp32, kind="ExternalInput")
    xsA_d = nc.dram_tensor("xsA", [C, HC + 2, RA], bf16, kind="ExternalInput")
    xsB_d = nc.dram_tensor("xsB", [C, HC + 2, RB], bf16, kind="ExternalInput")
    out_d = nc.dram_tensor("out", [C, HC, W], fp32, kind="ExternalOutput")

    with tile.TileContext(nc) as tc:
        with (
            tc.tile_pool(name="const", bufs=1) as cp,
            tc.tile_pool(name="work", bufs=1) as wp,
            tc.tile_pool(name="tmp", bufs=3) as tp,
            tc.tile_pool(name="psT", bufs=2, space="PSUM") as ppT,
            tc.tile_pool(name="rep", bufs=1, space="PSUM") as ppR,
            tc.tile_pool(name="psS", bufs=2, space="PSUM") as ppS,
        ):
            consts = cp.tile([C, 1416], bf16)
            bias = cp.tile([72, 1], fp32)
            xbfA = cp.tile([C, HC + 2, RA], bf16)
            xbfB = cp.tile([C, HC + 2, RB], bf16)

            lhsT = consts[:, 0:648]
            w4T = consts[:, 648:776]
            w8T = consts[0:32, 776:904]
            SA = consts[0:72, 904:1160]
            SB = consts[0:72, 1160:1416]

            nc.sync.dma_start(consts[:], consts_d[:])
            nc.sync.dma_start(xbfA[:, 0:9, :], xsA_d[:, 0:9, :])
            nc.sync.dma_start(xbfA[:, 9:30, :], xsA_d[:, 9:30, :])
            nc.sync.dma_start(bias[:], bias_d[:])
            nc.sync.dma_start(xbfB[:], xsB_d[:])

            PhA = wp.tile([C, NCH, CHN], bf16)
            PhB = wp.tile([C, NCH, CHN], bf16)
            samp = wp.tile([C, 9, NPIX], bf16)
            prod = wp.tile([C, 9, NPIX], bf16)
            v3t = wp.tile([C, 2, NHALF], bf16)
            res = wp.tile([C, 2, NHALF], fp32)

            for ch in range(NCH):
                # ---- T matmul (offset conv + affine expansion) ----
                psT = ppT.tile([72, CHN], fp32, tag="psT")
                for tap in range(9):
                    di, dj = tap // 3, tap % 3
                    rhs = xbfA[:, ch * CH + di : ch * CH + di + CH, dj : dj + W]
                    nc.tensor.matmul(
                        psT[:],
                        lhsT[:, tap * 72 : (tap + 1) * 72],
                        rhs,
                        start=(tap == 0),
                        stop=(tap == 8),
                    )
                # u = |T + bias|  (PSUM -> SBUF bf16)
                u = tp.tile([72, CHN], bf16, tag="u")
                nc.scalar.activation(u[:], psT[:], AF.Abs, bias=bias[:], scale=1.0)

                # ---- replication matmuls + fused relu drains ----
                YXA = tp.tile([C, 2, CHN], bf16, tag="yxa")
                YXB = tp.tile([C, 2, CHN], bf16, tag="yxb")
                UA = ppR.tile([C, 2, 512], fp32, tag="rep")
                nc.tensor.matmul(UA[:, 0, 0:CHN], SA[:, 0:128], u[:], start=True, stop=True)
                nc.tensor.matmul(UA[:, 1, 0:CHN], SA[:, 128:256], u[:], start=True, stop=True)
                nc.scalar.activation(YXA[:], UA[:, :, 0:CHN], AF.Relu, bias=1.0, scale=-1.0)
                UB = ppR.tile([C, 2, 512], fp32, tag="rep")
                nc.tensor.matmul(UB[:, 0, 0:CHN], SB[:, 0:128], u[:], start=True, stop=True)
                nc.tensor.matmul(UB[:, 1, 0:CHN], SB[:, 128:256], u[:], start=True, stop=True)
                nc.scalar.activation(YXB[:], UB[:, :, 0:CHN], AF.Relu, bias=1.0, scale=-1.0)

                # ---- Phi products (bf16 SBUF, 2x) ----
                nc.vector.tensor_mul(PhA[:, ch, :], YXA[:, 0, :], YXA[:, 1, :])
                nc.vector.tensor_mul(PhB[:, ch, :], YXB[:, 0, :], YXB[:, 1, :])

                # ---- samp matmuls, pair-packed PSUM + pair drains ----
                def samp_mm(k, dst_ps):
                    if k == 8:
                        nc.tensor.matmul(
                            dst_ps, w8T, PhA[0:32, ch, :],
                            start=True, stop=True, tile_position=(0, 0),
                        )
                    else:
                        g = k % 4
                        src = PhA if k < 4 else PhB
                        base = 32 * g
                        nc.tensor.matmul(
                            dst_ps,
                            w4T[base : base + 16, :],
                            src[base : base + 16, ch, :],
                            start=True, stop=True, tile_position=(base, 0),
                        )

                csl = slice(ch * CHN, (ch + 1) * CHN)
                for p in range(4):
                    psS = ppS.tile([C, 2, 512], fp32, tag="psS")
                    samp_mm(2 * p, psS[:, 0, 0:CHN])
                    samp_mm(2 * p + 1, psS[:, 1, 0:CHN])
                    dst = samp[:, 2 * p : 2 * p + 2, csl]
                    if p == 2:
                        nc.vector.tensor_copy(out=dst, in_=psS[:, :, 0:CHN])
                    else:
                        nc.scalar.copy(dst, psS[:, :, 0:CHN])
                psS = ppS.tile([C, 2, 512], fp32, tag="psS")
                samp_mm(8, psS[:, 0, 0:CHN])
                nc.scalar.copy(samp[:, 8, csl], psS[:, 0, 0:CHN])

                # ---- tail per half (after chunks 1 and 3) ----
                if ch % 2 == 1:
                    h = ch // 2
                    c0, c1 = h * NHALF, (h + 1) * NHALF
                    HH = NHALF // W  # 14 rows
                    r0 = h * HH
                    for k in range(9):
                        di, dj = k // 3, k % 3
                        if dj == 1:
                            xsrc, coff = xbfB, 2
                        else:
                            xsrc, coff = xbfA, dj
                        xv = xsrc[:, r0 + di : r0 + di + HH, coff : coff + W]
                        nc.vector.tensor_mul(
                            prod[:, k, c0:c1].rearrange("p (h w) -> p h w", h=HH),
                            samp[:, k, c0:c1].rearrange("p (h w) -> p h w", h=HH),
                            xv,
                        )
                    pv = prod[:, :, c0:c1]
                    # k-sum tree: levels 1-2 on DMA engines (accumulate DMA)
                    nc.gpsimd.dma_start(pv[:, 0, :], pv[:, 1, :], accum_op=ADD)
                    nc.gpsimd.dma_start(pv[:, 2, :], pv[:, 3, :], accum_op=ADD)
                    nc.gpsimd.dma_start(pv[:, 4, :], pv[:, 5, :], accum_op=ADD)
                    nc.gpsimd.dma_start(pv[:, 6, :], pv[:, 7, :], accum_op=ADD)
                    nc.gpsimd.dma_start(pv[:, 0, :], pv[:, 2, :], accum_op=ADD)
                    nc.gpsimd.dma_start(pv[:, 4, :], pv[:, 6, :], accum_op=ADD)
                    # final adds on DVE (last one casts to fp32)
                    nc.vector.tensor_add(v3t[:, h], pv[:, 0, :], pv[:, 4, :])
                    nc.vector.tensor_add(res[:, h], v3t[:, h], pv[:, 8, :])
                    nc.sync.dma_start(
                        out_d[:, r0 : r0 + HH, :],
                        res[:, h].rearrange("p (h w) -> p h w", h=HH),
                    )

    nc.finalize()
    return nc


def _prep_inputs(x, offset_w, offset_b, weight):
    """Host-side sharding + weight reshaping. Returns per-core input maps."""
    x = np.asarray(x, dtype=np.float32)
    offset_w = np.asarray(offset_w, dtype=np.float32)
    offset_b = np.asarray(offset_b, dtype=np.float32)
    weight = np.asarray(weight, dtype=np.float32)

    # lhsT[c, tap*72 + k*8 + axis*4 + i] = offset_w[2k+axis, c, tap//3, tap%3]
    ow = offset_w.reshape(9, 2, C, 3, 3)  # [k, axis, c, di, dj]
    lhsT = np.transpose(ow, (2, 3, 4, 0, 1))  # [c, di, dj, k, axis]
    lhsT = np.repeat(lhsT[..., None], 4, axis=-1)  # [c, di, dj, k, axis, i]
    lhsT = lhsT.reshape(C, 648)

    # w4T rows 32g+s = weight[:, s//4, s%4]; w8T rows 16..31 = same
    wT = weight.reshape(C, 16).T  # [16, C]
    w4T = np.zeros((C, 128), dtype=np.float32)
    for g in range(4):
        w4T[32 * g : 32 * g + 16, :] = wT
    w8T = np.zeros((C, 128), dtype=np.float32)
    w8T[16:32, :] = wT

    # selection matrices [72, 128]: row r = k*8 + axis*4 + i
    SAy = np.zeros((C, 128), dtype=np.float32)
    SAx = np.zeros((C, 128), dtype=np.float32)
    SBy = np.zeros((C, 128), dtype=np.float32)
    SBx = np.zeros((C, 128), dtype=np.float32)
    for k in range(4):
        for s in range(16):
            yi, xi = s // 4, s % 4
            SAy[k * 8 + yi, 32 * k + s] = 1.0
            SAx[k * 8 + 4 + xi, 32 * k + s] = 1.0
            SBy[(k + 4) * 8 + yi, 32 * k + s] = 1.0
            SBx[(k + 4) * 8 + 4 + xi, 32 * k + s] = 1.0
    # k=8 patterns ride in A rows 16..31
    for s in range(16):
        yi, xi = s // 4, s % 4
        SAy[64 + yi, 16 + s] = 1.0
        SAx[64 + 4 + xi, 16 + s] = 1.0

    consts = np.concatenate(
        [lhsT, w4T, w8T, np.concatenate([SAy, SAx], axis=1),
         np.concatenate([SBy, SBx], axis=1)], axis=1
    ).astype(_BF16)

    # bias72[k*8+axis*4+i] = offset_b[2k+axis] + base - i
    base = np.arange(3, dtype=np.float32) + 0.5
    bias = np.zeros((9, 2, 4), dtype=np.float32)
    for k in range(9):
        for axis in range(2):
            bv = base[k // 3] if axis == 0 else base[k % 3]
            bias[k, axis, :] = offset_b[2 * k + axis] + bv - np.arange(4)
    bias72 = bias.reshape(72, 1)

    xb = x.astype(_BF16)
    in_maps = []
    for core in range(8):
        b, half = core // 2, core % 2
        h0 = half * HC
        xsA = np.zeros((C, HC + 2, RA), dtype=_BF16)
        xsB = np.zeros((C, HC + 2, RB), dtype=_BF16)
        lo, hi = h0 - 1, h0 + HC + 1
        slo, shi = max(lo, 0), min(hi, H)
        xsA[:, slo - lo : slo - lo + (shi - slo), 1:57] = xb[b, :, slo:shi, :]
        xsB[:, slo - lo : slo - lo + (shi - slo), 2:58] = xb[b, :, slo:shi, :]
        in_maps.append(
            {"consts": consts, "bias72": bias72, "xsA": xsA, "xsB": xsB}
        )
    return in_maps


def kernel(x, offset_w, offset_b, weight):
    from concourse.bass_utils import run_bass_kernel_spmd

    if "nc" not in _cache:
        _cache["nc"] = _build_program()
    nc = _cache["nc"]

    in_maps = _prep_inputs(x, offset_w, offset_b, weight)
    res = run_bass_kernel_spmd(nc, in_maps, core_ids=list(range(8)))

    out = np.zeros((B, C, H, W), dtype=np.float32)
    for core in range(8):
        b, half = core // 2, core % 2
        out[b, :, half * HC : (half + 1) * HC, :] = res.results[core]["out"].reshape(
            C, HC, W
        )
    return out


# revision 4
# speedup vs baseline: 1.9003x; 1.1105x over previous
"""Trainium2 Bass kernel for nn_DeformKernelConv2d.

Math (per batch image; shapes below are per core after sharding):
  offsets:  off = conv3x3(x, offset_w) + offset_b          -> dy,dx per (k, pixel)
  coords:   yc_k = dy_k + by_k ; xc_k = dx_k + bx_k        (scope-kernel space)
  phi:      phi_y[k,i] = relu(1-|yc_k - i|), i=0..3        (likewise phi_x)
  Phi:      Phi_k[4*yi+xi] = phi_y[k,yi] * phi_x[k,xi]     (bilinear weights, 16 per k)
  samp:     samp_k[c] = sum_s wflat[c,s] * Phi_k[s]        (matmul over s=16)
  out:      out[c] = sum_k samp_k[c] * x_k[c]              (x_k = 3x3-shifted x)

Device mapping (v4):
  - 8 cores: (batch b, H-half); each core does 28 rows with a 1-row halo.
  - offset conv fused with the affine expansion into a 9-tap accumulated
    matmul -> T[72, pix]; u = |T + bias| in one ACT op.
  - row replication of u into 128-row (k,s) patterns via PE matmuls against
    0/1 selection matrices; k=8 rides in rows 16..31 of the A tile and its
    samp matmul contracts rows 0..31 against a half-zero [32,128] weight.
  - phi = relu(1 - u) fused into the PSUM->SBUF drains (ACT).
  - chunk loop is software-pipelined: chunk ch's taps are emitted before
    chunk ch-1's phi/samp stages so the PE always has a dense run of
    matmuls (keeps the HAM clock gate warm).
  - samp matmuls pair-packed into 2-bank PSUM tiles, drained 2 k-planes per
    instruction, split ACT/DVE.
  - tail per pixel-half: 9 prod muls on DVE interleaved with 4 accumulate-
    DMA partial sums on the idle DMA engines; final adds on DVE.
"""

import numpy as np
import ml_dtypes

B, C, H, W = 4, 128, 56, 56
HC = H // 2            # 28 rows per core
NPIX = HC * W          # 1568
CH = 7                 # chunk height (rows)
NCH = HC // CH         # 4 chunks
CHN = CH * W           # 392 columns per chunk
RA, RB = 58, 60        # padded row lengths: xbfA data at col 1, xbfB at col 2
NHALF = NPIX // 2      # 784 (14 rows) per tail half

_BF16 = ml_dtypes.bfloat16
_cache = {}


def _build_program():
    import concourse.tile as tile
    import concourse.mybir as mybir
    from concourse import bacc

    fp32 = mybir.dt.float32
    bf16 = mybir.dt.bfloat16
    AF = mybir.ActivationFunctionType
    ADD = mybir.AluOpType.add

    nc = bacc.Bacc("TRN2", target_bir_lowering=False, debug=False, num_devices=8)
    consts_d = nc.dram_tensor("consts", [C, 1416], bf16, kind="ExternalInput")
    bias_d = nc.dram_tensor("bias72", [72, 1], fp32, kind="ExternalInput")
    xsA_d = nc.dram_tensor("xsA", [C, HC + 2, RA], bf16, kind="ExternalInput")
    xsB_d = nc.dram_tensor("xsB", [C, HC + 2, RB], bf16, kind="ExternalInput")
    out_d = nc.dram_tensor("out", [C, HC, W], fp32, kind="ExternalOutput")

    with tile.TileContext(nc) as tc:
        with (
            tc.tile_pool(name="const", bufs=1) as cp,
            tc.tile_pool(name="work", bufs=1) as wp,
            tc.tile_pool(name="tmp", bufs=3) as tp,
            tc.tile_pool(name="psT", bufs=2, space="PSUM") as ppT,
            tc.tile_pool(name="rep", bufs=1, space="PSUM") as ppR,
            tc.tile_pool(name="psS", bufs=2, space="PSUM") as ppS,
        ):
            consts = cp.tile([C, 1416], bf16)
            bias = cp.tile([72, 1], fp32)
            xbfA = cp.tile([C, HC + 2, RA], bf16)
            xbfB = cp.tile([C, HC + 2, RB], bf16)

            lhsT = consts[:, 0:648]
            w4T = consts[:, 648:776]
            w8T = consts[0:32, 776:904]
            SA = consts[0:72, 904:1160]
            SB = consts[0:72, 1160:1416]

            # split loads across the two HWDGE queues so they don't serialize
            nc.sync.dma_start(consts[:], consts_d[:])
            nc.sync.dma_start(xbfA[:, 0:9, :], xsA_d[:, 0:9, :])
            nc.scalar.dma_start(xbfA[:, 9:30, :], xsA_d[:, 9:30, :])
            nc.scalar.dma_start(bias[:], bias_d[:])
            nc.scalar.dma_start(xbfB[:], xsB_d[:])

            PhA = wp.tile([C, NCH, CHN], bf16)
            PhB = wp.tile([C, NCH, CHN], bf16)
            samp = wp.tile([C, 9, NPIX], bf16)
            prod = wp.tile([C, 9, NPIX], bf16)
            v4a = wp.tile([C, 2, NHALF], bf16)
            v4b = wp.tile([C, 2, NHALF], bf16)
            v4c = wp.tile([C, 2, NHALF], bf16)
            res = wp.tile([C, 2, NHALF], fp32)

            psT_tiles = {}
            u_tiles = {}

            def emit_taps(ch):
                psT = ppT.tile([72, CHN], fp32, tag="psT")
                psT_tiles[ch] = psT
                for tap in range(9):
                    di, dj = tap // 3, tap % 3
                    rhs = xbfA[:, ch * CH + di : ch * CH + di + CH, dj : dj + W]
                    nc.tensor.matmul(
                        psT[:],
                        lhsT[:, tap * 72 : (tap + 1) * 72],
                        rhs,
                        start=(tap == 0),
                        stop=(tap == 8),
                    )
                u = tp.tile([72, CHN], bf16, tag="u")
                u_tiles[ch] = u
                nc.scalar.activation(u[:], psT[:], AF.Abs, bias=bias[:], scale=1.0)

            def emit_phi_samp(ch):
                u = u_tiles.pop(ch)
                YXA = tp.tile([C, 2, CHN], bf16, tag="yxa")
                YXB = tp.tile([C, 2, CHN], bf16, tag="yxb")
                UA = ppR.tile([C, 2, 512], fp32, tag="rep")
                nc.tensor.matmul(UA[:, 0, 0:CHN], SA[:, 0:128], u[:], start=True, stop=True)
                nc.tensor.matmul(UA[:, 1, 0:CHN], SA[:, 128:256], u[:], start=True, stop=True)
                nc.scalar.activation(YXA[:], UA[:, :, 0:CHN], AF.Relu, bias=1.0, scale=-1.0)
                UB = ppR.tile([C, 2, 512], fp32, tag="rep")
                nc.tensor.matmul(UB[:, 0, 0:CHN], SB[:, 0:128], u[:], start=True, stop=True)
                nc.tensor.matmul(UB[:, 1, 0:CHN], SB[:, 128:256], u[:], start=True, stop=True)
                nc.scalar.activation(YXB[:], UB[:, :, 0:CHN], AF.Relu, bias=1.0, scale=-1.0)

                nc.vector.tensor_mul(PhA[:, ch, :], YXA[:, 0, :], YXA[:, 1, :])
                nc.vector.tensor_mul(PhB[:, ch, :], YXB[:, 0, :], YXB[:, 1, :])

                def samp_mm(k, dst_ps):
                    if k == 8:
                        nc.tensor.matmul(
                            dst_ps, w8T, PhA[0:32, ch, :],
                            start=True, stop=True, tile_position=(0, 0),
                        )
                    else:
                        g = k % 4
                        src = PhA if k < 4 else PhB
                        base = 32 * g
                        nc.tensor.matmul(
                            dst_ps,
                            w4T[base : base + 16, :],
                            src[base : base + 16, ch, :],
                            start=True, stop=True, tile_position=(base, 0),
                        )

                csl = slice(ch * CHN, (ch + 1) * CHN)
                for p in range(4):
                    psS = ppS.tile([C, 2, 512], fp32, tag="psS")
                    samp_mm(2 * p, psS[:, 0, 0:CHN])
                    samp_mm(2 * p + 1, psS[:, 1, 0:CHN])
                    dst = samp[:, 2 * p : 2 * p + 2, csl]
                    if p == (1 if ch % 2 == 0 else 2):
                        nc.vector.tensor_copy(out=dst, in_=psS[:, :, 0:CHN])
                    else:
                        nc.scalar.copy(dst, psS[:, :, 0:CHN])
                psS = ppS.tile([C, 2, 512], fp32, tag="psS")
                samp_mm(8, psS[:, 0, 0:CHN])
                nc.scalar.copy(samp[:, 8, csl], psS[:, 0, 0:CHN])

            def emit_tail(h):
                c0, c1 = h * NHALF, (h + 1) * NHALF
                HH = NHALF // W  # 14 rows
                r0 = h * HH
                pv = prod[:, :, c0:c1]

                def mul(k):
                    di, dj = k // 3, k % 3
                    if dj == 1:
                        xsrc, coff = xbfB, 2
                    else:
                        xsrc, coff = xbfA, dj
                    xv = xsrc[:, r0 + di : r0 + di + HH, coff : coff + W]
                    nc.vector.tensor_mul(
                        prod[:, k, c0:c1].rearrange("p (h w) -> p h w", h=HH),
                        samp[:, k, c0:c1].rearrange("p (h w) -> p h w", h=HH),
                        xv,
                    )

                # interleave DVE muls with level-1 accumulate DMAs
                for p in range(4):
                    mul(2 * p)
                    mul(2 * p + 1)
                    nc.gpsimd.dma_start(pv[:, 2 * p, :], pv[:, 2 * p + 1, :], accum_op=ADD)
                mul(8)
                # final adds on DVE (last one casts to fp32)
                nc.vector.tensor_add(v4a[:, h], pv[:, 0, :], pv[:, 2, :])
                nc.vector.tensor_add(v4b[:, h], pv[:, 4, :], pv[:, 6, :])
                nc.vector.tensor_add(v4c[:, h], v4a[:, h], v4b[:, h])
                nc.vector.tensor_add(res[:, h], v4c[:, h], pv[:, 8, :])
                nc.sync.dma_start(
                    out_d[:, r0 : r0 + HH, :],
                    res[:, h].rearrange("p (h w) -> p h w", h=HH),
                )

            # software pipeline: taps(ch) issue ahead of phi/samp(ch-1)
            for ch in range(NCH + 1):
                if ch < NCH:
                    emit_taps(ch)
                if ch >= 1:
                    emit_phi_samp(ch - 1)
                    if (ch - 1) % 2 == 1:
                        emit_tail((ch - 1) // 2)

    nc.finalize()
    return nc


def _prep_inputs(x, offset_w, offset_b, weight):
    """Host-side sharding + weight reshaping. Returns per-core input maps."""
    x = np.asarray(x, dtype=np.float32)
    offset_w = np.asarray(offset_w, dtype=np.float32)
    offset_b = np.asarray(offset_b, dtype=np.float32)
    weight = np.asarray(weight, dtype=np.float32)

    # lhsT[c, tap*72 + k*8 + axis*4 + i] = offset_w[2k+axis, c, tap//3, tap%3]
    ow = offset_w.reshape(9, 2, C, 3, 3)  # [k, axis, c, di, dj]
    lhsT = np.transpose(ow, (2, 3, 4, 0, 1))  # [c, di, dj, k, axis]
    lhsT = np.repeat(lhsT[..., None], 4, axis=-1)  # [c, di, dj, k, axis, i]
    lhsT = lhsT.reshape(C, 648)

    # w4T rows 32g+s = weight[:, s//4, s%4]; w8T rows 16..31 = same
    wT = weight.reshape(C, 16).T  # [16, C]
    w4T = np.zeros((C, 128), dtype=np.float32)
    for g in range(4):
        w4T[32 * g : 32 * g + 16, :] = wT
    w8T = np.zeros((C, 128), dtype=np.float32)
    w8T[16:32, :] = wT

    # selection matrices [72, 128]: row r = k*8 + axis*4 + i
    SAy = np.zeros((C, 128), dtype=np.float32)
    SAx = np.zeros((C, 128), dtype=np.float32)
    SBy = np.zeros((C, 128), dtype=np.float32)
    SBx = np.zeros((C, 128), dtype=np.float32)
    for k in range(4):
        for s in range(16):
            yi, xi = s // 4, s % 4
            SAy[k * 8 + yi, 32 * k + s] = 1.0
            SAx[k * 8 + 4 + xi, 32 * k + s] = 1.0
            SBy[(k + 4) * 8 + yi, 32 * k + s] = 1.0
            SBx[(k + 4) * 8 + 4 + xi, 32 * k + s] = 1.0
    # k=8 patterns ride in A rows 16..31
    for s in range(16):
        yi, xi = s // 4, s % 4
        SAy[64 + yi, 16 + s] = 1.0
        SAx[64 + 4 + xi, 16 + s] = 1.0

    consts = np.concatenate(
        [lhsT, w4T, w8T, np.concatenate([SAy, SAx], axis=1),
         np.concatenate([SBy, SBx], axis=1)], axis=1
    ).astype(_BF16)

    # bias72[k*8+axis*4+i] = offset_b[2k+axis] + base - i
    base = np.arange(3, dtype=np.float32) + 0.5
    bias = np.zeros((9, 2, 4), dtype=np.float32)
    for k in range(9):
        for axis in range(2):
            bv = base[k // 3] if axis == 0 else base[k % 3]
            bias[k, axis, :] = offset_b[2 * k + axis] + bv - np.arange(4)
    bias72 = bias.reshape(72, 1)

    xb = x.astype(_BF16)
    in_maps = []
    for core in range(8):
        b, half = core // 2, core % 2
        h0 = half * HC
        xsA = np.zeros((C, HC + 2, RA), dtype=_BF16)
        xsB = np.zeros((C, HC + 2, RB), dtype=_BF16)
        lo, hi = h0 - 1, h0 + HC + 1
        slo, shi = max(lo, 0), min(hi, H)
        xsA[:, slo - lo : slo - lo + (shi - slo), 1:57] = xb[b, :, slo:shi, :]
        xsB[:, slo - lo : slo - lo + (shi - slo), 2:58] = xb[b, :, slo:shi, :]
        in_maps.append(
            {"consts": consts, "bias72": bias72, "xsA": xsA, "xsB": xsB}
        )
    return in_maps


def kernel(x, offset_w, offset_b, weight):
    from concourse.bass_utils import run_bass_kernel_spmd

    if "nc" not in _cache:
        _cache["nc"] = _build_program()
    nc = _cache["nc"]

    in_maps = _prep_inputs(x, offset_w, offset_b, weight)
    res = run_bass_kernel_spmd(nc, in_maps, core_ids=list(range(8)))

    out = np.zeros((B, C, H, W), dtype=np.float32)
    for core in range(8):
        b, half = core // 2, core % 2
        out[b, :, half * HC : (half + 1) * HC, :] = res.results[core]["out"].reshape(
            C, HC, W
        )
    return out


# revision 5
# speedup vs baseline: 1.9458x; 1.0240x over previous
"""Trainium2 Bass kernel for nn_DeformKernelConv2d.

Math (per batch image; shapes below are per core after sharding):
  offsets:  off = conv3x3(x, offset_w) + offset_b          -> dy,dx per (k, pixel)
  coords:   yc_k = dy_k + by_k ; xc_k = dx_k + bx_k        (scope-kernel space)
  phi:      phi_y[k,i] = relu(1-|yc_k - i|), i=0..3        (likewise phi_x)
  Phi:      Phi_k[4*yi+xi] = phi_y[k,yi] * phi_x[k,xi]     (bilinear weights, 16 per k)
  samp:     samp_k[c] = sum_s wflat[c,s] * Phi_k[s]        (matmul over s=16)
  out:      out[c] = sum_k samp_k[c] * x_k[c]              (x_k = 3x3-shifted x)

Device mapping (v5):
  - 8 cores: (batch b, H-half); each core does 28 rows with a 1-row halo.
  - offset conv fused with the affine expansion into a 9-tap accumulated
    matmul -> T[72, pix]; u = |T + bias| in one ACT op.
  - row replication of u into 128-row (k,s) patterns via PE matmuls against
    0/1 selection matrices; k=8 rides in rows 16..31 of the A tile and its
    samp matmul contracts rows 0..31 against a half-zero [32,128] weight.
  - phi = relu(1 - u) fused into the PSUM->SBUF drains (ACT).
  - chunk rows are uneven (9/9/8/2): big chunks amortize per-chunk matmul
    overhead and give the PE long dense bursts; the last chunk is tiny so
    the end-exposed tail is short.
  - software pipelined: chunk ch's taps are emitted before chunk ch-1's
    phi/samp stages.
  - samp matmuls pair-packed into 2-bank PSUM tiles, drained 2 k-planes per
    ACT instruction; tail (prod mul + 9-way k-sum tree) entirely on DVE.
"""

import numpy as np
import ml_dtypes

B, C, H, W = 4, 128, 56, 56
HC = H // 2            # 28 rows per core
NPIX = HC * W          # 1568
RA, RB = 58, 60        # padded row lengths: xbfA data at col 1, xbfB at col 2

CH_ROWS = [9, 9, 8, 2]
CH_R0 = [0, 9, 18, 26]
NCH = 4
# tails: (emit-after-chunk, pixel range, row range)
TAILS = [(1, 0, 18), (2, 18, 26), (3, 26, 28)]

_BF16 = ml_dtypes.bfloat16
_cache = {}


def _build_program():
    import concourse.tile as tile
    import concourse.mybir as mybir
    from concourse import bacc

    fp32 = mybir.dt.float32
    bf16 = mybir.dt.bfloat16
    AF = mybir.ActivationFunctionType

    nc = bacc.Bacc("TRN2", target_bir_lowering=False, debug=False, num_devices=8)
    lhsT_d = nc.dram_tensor("lhsT", [C, 648], bf16, kind="ExternalInput")
    # consts2 cols: [0:128] w4T | [128:256] w8T(rows<32) | [256:512] SA | [512:768] SB
    consts2_d = nc.dram_tensor("consts2", [C, 768], bf16, kind="ExternalInput")
    bias_d = nc.dram_tensor("bias72", [72, 1], fp32, kind="ExternalInput")
    xsA_d = nc.dram_tensor("xsA", [C, HC + 2, RA], bf16, kind="ExternalInput")
    xsB_d = nc.dram_tensor("xsB", [C, HC + 2, RB], bf16, kind="ExternalInput")
    out_d = nc.dram_tensor("out", [C, HC, W], fp32, kind="ExternalOutput")

    with tile.TileContext(nc) as tc:
        with (
            tc.tile_pool(name="const", bufs=1) as cp,
            tc.tile_pool(name="work", bufs=1) as wp,
            tc.tile_pool(name="tmp", bufs=2) as tp,
            tc.tile_pool(name="psT", bufs=2, space="PSUM") as ppT,
            tc.tile_pool(name="rep", bufs=1, space="PSUM") as ppR,
            tc.tile_pool(name="psS", bufs=2, space="PSUM") as ppS,
        ):
            lhsT = cp.tile([C, 648], bf16)
            consts2 = cp.tile([C, 768], bf16)
            bias = cp.tile([72, 1], fp32)
            xbfA = cp.tile([C, HC + 2, RA], bf16)
            xbfB = cp.tile([C, HC + 2, RB], bf16)

            w4T = consts2[:, 0:128]
            w8T = consts2[0:32, 128:256]
            SA = consts2[0:72, 256:512]
            SB = consts2[0:72, 512:768]

            # loads split across the two HWDGE queues; one DMA per tile so
            # consumers gate on exactly the data they need
            nc.sync.dma_start(bias[:], bias_d[:])
            nc.sync.dma_start(xbfA[:], xsA_d[:])
            nc.scalar.dma_start(lhsT[:], lhsT_d[:])
            nc.scalar.dma_start(consts2[:], consts2_d[:])
            nc.scalar.dma_start(xbfB[:], xsB_d[:])

            samp = wp.tile([C, 9, NPIX], bf16)
            prod = wp.tile([C, 9, NPIX], bf16)

            u_tiles = {}
            psT_tiles = {}

            def emit_taps(ch):
                R = CH_ROWS[ch]
                r0 = CH_R0[ch]
                N = R * W
                psT = ppT.tile([72, 512], fp32, tag="psT")
                psT_tiles[ch] = psT
                for tap in range(9):
                    di, dj = tap // 3, tap % 3
                    rhs = xbfA[:, r0 + di : r0 + di + R, dj : dj + W]
                    nc.tensor.matmul(
                        psT[:, 0:N],
                        lhsT[:, tap * 72 : (tap + 1) * 72],
                        rhs,
                        start=(tap == 0),
                        stop=(tap == 8),
                    )
                u = tp.tile([72, 512], bf16, tag="u")
                u_tiles[ch] = u
                nc.scalar.activation(
                    u[:, 0:N], psT[:, 0:N], AF.Abs, bias=bias[:], scale=1.0
                )

            def emit_phi_samp(ch):
                R = CH_ROWS[ch]
                N = R * W
                c0 = CH_R0[ch] * W
                u = u_tiles.pop(ch)
                YXA = tp.tile([C, 2, 512], bf16, tag="yxa")
                YXB = tp.tile([C, 2, 512], bf16, tag="yxb")
                PhA = tp.tile([C, 512], bf16, tag="phA")
                PhB = tp.tile([C, 512], bf16, tag="phB")
                UA = ppR.tile([C, 2, 512], fp32, tag="rep")
                nc.tensor.matmul(UA[:, 0, 0:N], SA[:, 0:128], u[:, 0:N], start=True, stop=True)
                nc.tensor.matmul(UA[:, 1, 0:N], SA[:, 128:256], u[:, 0:N], start=True, stop=True)
                nc.scalar.activation(YXA[:, :, 0:N], UA[:, :, 0:N], AF.Relu, bias=1.0, scale=-1.0)
                UB = ppR.tile([C, 2, 512], fp32, tag="rep")
                nc.tensor.matmul(UB[:, 0, 0:N], SB[:, 0:128], u[:, 0:N], start=True, stop=True)
                nc.tensor.matmul(UB[:, 1, 0:N], SB[:, 128:256], u[:, 0:N], start=True, stop=True)
                nc.scalar.activation(YXB[:, :, 0:N], UB[:, :, 0:N], AF.Relu, bias=1.0, scale=-1.0)

                nc.vector.tensor_mul(PhA[:, 0:N], YXA[:, 0, 0:N], YXA[:, 1, 0:N])
                nc.vector.tensor_mul(PhB[:, 0:N], YXB[:, 0, 0:N], YXB[:, 1, 0:N])

                def samp_mm(k, dst_ps):
                    if k == 8:
                        nc.tensor.matmul(
                            dst_ps, w8T, PhA[0:32, 0:N],
                            start=True, stop=True, tile_position=(0, 0),
                        )
                    else:
                        g = k % 4
                        src = PhA if k < 4 else PhB
                        base = 32 * g
                        nc.tensor.matmul(
                            dst_ps,
                            w4T[base : base + 16, :],
                            src[base : base + 16, 0:N],
                            start=True, stop=True, tile_position=(base, 0),
                        )

                csl = slice(c0, c0 + N)
                for p in range(4):
                    psS = ppS.tile([C, 2, 512], fp32, tag="psS")
                    samp_mm(2 * p, psS[:, 0, 0:N])
                    samp_mm(2 * p + 1, psS[:, 1, 0:N])
                    nc.scalar.copy(samp[:, 2 * p : 2 * p + 2, csl], psS[:, :, 0:N])
                psS = ppS.tile([C, 2, 512], fp32, tag="psS")
                samp_mm(8, psS[:, 0, 0:N])
                nc.scalar.copy(samp[:, 8, csl], psS[:, 0, 0:N])

            def emit_tail(row0, row1):
                c0, c1 = row0 * W, row1 * W
                HH = row1 - row0
                pv = prod[:, :, c0:c1]
                for k in range(9):
                    di, dj = k // 3, k % 3
                    if dj == 1:
                        xsrc, coff = xbfB, 2
                    else:
                        xsrc, coff = xbfA, dj
                    xv = xsrc[:, row0 + di : row0 + di + HH, coff : coff + W]
                    nc.vector.tensor_mul(
                        prod[:, k, c0:c1].rearrange("p (h w) -> p h w", h=HH),
                        samp[:, k, c0:c1].rearrange("p (h w) -> p h w", h=HH),
                        xv,
                    )
                t1a = tp.tile([C, 2, 1008], bf16, tag="t1a")
                t1b = tp.tile([C, 2, 1008], bf16, tag="t1b")
                t2 = tp.tile([C, 2, 1008], bf16, tag="t2")
                t3 = tp.tile([C, 1008], bf16, tag="t3")
                res = tp.tile([C, 1008], fp32, tag="res")
                N = c1 - c0
                nc.vector.tensor_add(t1a[:, :, 0:N], pv[:, 0:2, :], pv[:, 2:4, :])
                nc.vector.tensor_add(t1b[:, :, 0:N], pv[:, 4:6, :], pv[:, 6:8, :])
                nc.vector.tensor_add(t2[:, :, 0:N], t1a[:, :, 0:N], t1b[:, :, 0:N])
                nc.vector.tensor_add(t3[:, 0:N], t2[:, 0, 0:N], t2[:, 1, 0:N])
                nc.vector.tensor_add(res[:, 0:N], t3[:, 0:N], pv[:, 8, :])
                nc.sync.dma_start(
                    out_d[:, row0:row1, :],
                    res[:, 0:N].rearrange("p (h w) -> p h w", h=HH),
                )

            tails = {after: (r0, r1) for (after, r0, r1) in TAILS}
            for ch in range(NCH + 1):
                if ch < NCH:
                    emit_taps(ch)
                if ch >= 1:
                    emit_phi_samp(ch - 1)
                    if ch - 1 in tails:
                        emit_tail(*tails[ch - 1])

    nc.finalize()
    return nc


def _prep_inputs(x, offset_w, offset_b, weight):
    """Host-side sharding + weight reshaping. Returns per-core input maps."""
    x = np.asarray(x, dtype=np.float32)
    offset_w = np.asarray(offset_w, dtype=np.float32)
    offset_b = np.asarray(offset_b, dtype=np.float32)
    weight = np.asarray(weight, dtype=np.float32)

    # lhsT[c, tap*72 + k*8 + axis*4 + i] = offset_w[2k+axis, c, tap//3, tap%3]
    ow = offset_w.reshape(9, 2, C, 3, 3)  # [k, axis, c, di, dj]
    lhsT = np.transpose(ow, (2, 3, 4, 0, 1))  # [c, di, dj, k, axis]
    lhsT = np.repeat(lhsT[..., None], 4, axis=-1)  # [c, di, dj, k, axis, i]
    lhsT = np.ascontiguousarray(lhsT.reshape(C, 648)).astype(_BF16)

    # w4T rows 32g+s = weight[:, s//4, s%4]; w8T rows 16..31 = same
    wT = weight.reshape(C, 16).T  # [16, C]
    w4T = np.zeros((C, 128), dtype=np.float32)
    for g in range(4):
        w4T[32 * g : 32 * g + 16, :] = wT
    w8T = np.zeros((C, 128), dtype=np.float32)
    w8T[16:32, :] = wT

    # selection matrices [72, 128]: row r = k*8 + axis*4 + i
    SAy = np.zeros((C, 128), dtype=np.float32)
    SAx = np.zeros((C, 128), dtype=np.float32)
    SBy = np.zeros((C, 128), dtype=np.float32)
    SBx = np.zeros((C, 128), dtype=np.float32)
    for k in range(4):
        for s in range(16):
            yi, xi = s // 4, s % 4
            SAy[k * 8 + yi, 32 * k + s] = 1.0
            SAx[k * 8 + 4 + xi, 32 * k + s] = 1.0
            SBy[(k + 4) * 8 + yi, 32 * k + s] = 1.0
            SBx[(k + 4) * 8 + 4 + xi, 32 * k + s] = 1.0
    # k=8 patterns ride in A rows 16..31
    for s in range(16):
        yi, xi = s // 4, s % 4
        SAy[64 + yi, 16 + s] = 1.0
        SAx[64 + 4 + xi, 16 + s] = 1.0

    consts2 = np.concatenate(
        [w4T, w8T, np.concatenate([SAy, SAx], axis=1),
         np.concatenate([SBy, SBx], axis=1)], axis=1
    ).astype(_BF16)

    # bias72[k*8+axis*4+i] = offset_b[2k+axis] + base - i
    base = np.arange(3, dtype=np.float32) + 0.5
    bias = np.zeros((9, 2, 4), dtype=np.float32)
    for k in range(9):
        for axis in range(2):
            bv = base[k // 3] if axis == 0 else base[k % 3]
            bias[k, axis, :] = offset_b[2 * k + axis] + bv - np.arange(4)
    bias72 = bias.reshape(72, 1)

    xb = x.astype(_BF16)
    in_maps = []
    for core in range(8):
        b, half = core // 2, core % 2
        h0 = half * HC
        xsA = np.zeros((C, HC + 2, RA), dtype=_BF16)
        xsB = np.zeros((C, HC + 2, RB), dtype=_BF16)
        lo, hi = h0 - 1, h0 + HC + 1
        slo, shi = max(lo, 0), min(hi, H)
        xsA[:, slo - lo : slo - lo + (shi - slo), 1:57] = xb[b, :, slo:shi, :]
        xsB[:, slo - lo : slo - lo + (shi - slo), 2:58] = xb[b, :, slo:shi, :]
        in_maps.append(
            {"lhsT": lhsT, "consts2": consts2, "bias72": bias72,
             "xsA": xsA, "xsB": xsB}
        )
    return in_maps


def kernel(x, offset_w, offset_b, weight):
    from concourse.bass_utils import run_bass_kernel_spmd

    if "nc" not in _cache:
        _cache["nc"] = _build_program()
    nc = _cache["nc"]

    in_maps = _prep_inputs(x, offset_w, offset_b, weight)
    res = run_bass_kernel_spmd(nc, in_maps, core_ids=list(range(8)))

    out = np.zeros((B, C, H, W), dtype=np.float32)
    for core in range(8):
        b, half = core // 2, core % 2
        out[b, :, half * HC : (half + 1) * HC, :] = res.results[core]["out"].reshape(
            C, HC, W
        )
    return out
